# revision 13
# baseline (speedup 1.0000x reference)
"""Trainium2 Bass kernel for nn_C2fPSA (quaternion C2fPSA block).

Sharding: the 8 (batch, quaternion) slices are independent except for the 6
BatchNorm statistics, which are synced with small AllGathers + local reduce.
Each core processes one (b, q) slice of shape [C, 24, 24] in channel-major
[C, n=576] layout; all convs run on the TensorEngine (1x1 convs as matmuls,
3x3 convs as 9 shifted accumulating matmuls, depthwise 3x3 as diagonal-matrix
matmuls).  Attention: 16 heads of dim 16, head channels zero-padded to 32 so
QK^T can use 4-way tile_position row tiling; softmax is computed max-free
(scores are ~N(0, 0.05)); denominators come from a leading ones-column in the
augmented V operand; normalization uses a PE indicator-matmul broadcast.
"""
import numpy as np

NCORES = 8
P = 128
N = 576          # 24*24 spatial tokens per (b, q) slice
NH2 = 288        # free-dim half (psum bank = 512 f32; halves at +0 / +512)
EPS = 1e-5
MCNTS = [128, 128, 128, 128, 64]   # m-chunk sizes for 576 tokens

_CACHE = {}


def _build():
    import concourse.bacc as bacc
    import concourse.mybir as mybir
    import concourse.tile as tile

    F32 = mybir.dt.float32
    AF = mybir.ActivationFunctionType
    OP = mybir.AluOpType

    nc = bacc.Bacc("TRN2", target_bir_lowering=False, debug=False,
                   num_devices=NCORES)

    # ---------------- DRAM I/O ----------------
    din = {}
    def dram_in(name, shape):
        din[name] = nc.dram_tensor(name, list(shape), F32, kind="ExternalInput")
        return din[name]

    x_d = dram_in("x_s", (512, N))
    g_d = dram_in("gvec", (P, 1))
    w1_d = dram_in("w1t", (512, 512))
    wq_d = dram_in("wqt", (256, 512))      # head-padded co' (32/head)
    wk_d = dram_in("wkt", (256, 512))
    wv_d = dram_in("wvt", (256, 256))
    wa_d = dram_in("wat_pad", (512, 256))  # ci' padded (ones/d layout)
    pe_d = dram_in("pe_w", (256, 9))
    wf1_d = dram_in("wf1t", (256, 512))
    wf2_d = dram_in("wf2t", (512, 256))
    wec_d = dram_in("wect", (256, 128))
    wmp_d = dram_in("wmpt", (9, 128, 256))
    w2_d = dram_in("w2t", (1024, 512))
    id_d = dram_in("id128", (P, P))
    e4_d = dram_in("e4", (4, P))           # E4[j, 32j+1+d] = 1 (d<16)
    out_d = nc.dram_tensor("out", [512, N], F32, kind="ExternalOutput")

    with tile.TileContext(nc) as tc:
        import contextlib
        ctx = contextlib.ExitStack()
        with ctx:
            sb = ctx.enter_context(tc.tile_pool(name="sb", bufs=1))
            est_pool = ctx.enter_context(tc.tile_pool(name="est", bufs=4))
            avsb_pool = ctx.enter_context(tc.tile_pool(name="avsb", bufs=2))
            small = ctx.enter_context(tc.tile_pool(name="small", bufs=2))
            ps_conv = ctx.enter_context(
                tc.tile_pool(name="ps_conv", bufs=2, space="PSUM"))
            ps_av = ctx.enter_context(
                tc.tile_pool(name="ps_av", bufs=1, space="PSUM"))
            ps_rb = ctx.enter_context(
                tc.tile_pool(name="ps_rb", bufs=1, space="PSUM"))
            dram = ctx.enter_context(
                tc.tile_pool(name="dram", bufs=1, space="DRAM"))

            # ---------------- persistent SBUF ----------------
            x_sb = sb.tile([P, 4, N], F32)
            gvec = sb.tile([P, 1], F32)
            w1t = sb.tile([P, 4, 512], F32)
            wqt = sb.tile([P, 2, 512], F32)
            wkt = sb.tile([P, 2, 512], F32)
            wvt = sb.tile([P, 2, 256], F32)
            wat = sb.tile([P, 4, 256], F32)
            pew = sb.tile([P, 2, 9], F32)
            wf1t = sb.tile([P, 2, 512], F32)
            wf2t = sb.tile([P, 4, 256], F32)
            wect = sb.tile([P, 2, 128], F32)
            wmpt = sb.tile([P, 9, 256], F32)
            w2t = sb.tile([P, 8, 512], F32)
            id128 = sb.tile([P, P], F32)
            e4t = sb.tile([4, P], F32)

            def ld(dst, src):
                nc.sync.dma_start(dst, src)

            ld(x_sb[:], x_d[:].rearrange("(a p) f -> p a f", p=P))
            ld(gvec[:], g_d[:])
            ld(w1t[:], w1_d[:].rearrange("(a p) f -> p a f", p=P))
            ld(wqt[:], wq_d[:].rearrange("(a p) f -> p a f", p=P))
            ld(wkt[:], wk_d[:].rearrange("(a p) f -> p a f", p=P))
            ld(wvt[:], wv_d[:].rearrange("(a p) f -> p a f", p=P))
            ld(wat[:], wa_d[:].rearrange("(a p) f -> p a f", p=P))
            ld(pew[:], pe_d[:].rearrange("(a p) f -> p a f", p=P))
            ld(wf1t[:], wf1_d[:].rearrange("(a p) f -> p a f", p=P))
            ld(wf2t[:], wf2_d[:].rearrange("(a p) f -> p a f", p=P))
            ld(wect[:], wec_d[:].rearrange("(a p) f -> p a f", p=P))
            ld(wmpt[:], wmp_d[:].transpose([1, 0, 2]))
            ld(w2t[:], w2_d[:].rearrange("(a p) f -> p a f", p=P))
            ld(id128[:], id_d[:])
            ld(e4t[:], e4_d[:])

            # consts
            ones_row = sb.tile([1, P], F32)
            ones_col = sb.tile([P, 1], F32)
            zc = sb.tile([1, P], F32)
            zr = sb.tile([1, NH2], F32)
            nc.vector.memset(ones_row[:], 1.0)
            nc.vector.memset(ones_col[:], 1.0)
            nc.vector.memset(zc[:], 0.0)
            nc.vector.memset(zr[:], 0.0)

            # ACT table prewarm (exp/ln set)
            junk1 = small.tile([1, 1], F32, tag="junk1")
            nc.scalar.activation(junk1[:], ones_row[0:1, 0:1], AF.Exp)
            junk2 = small.tile([1, 1], F32, tag="junk1")
            nc.scalar.activation(junk2[:], ones_row[0:1, 0:1], AF.Ln)

            # activations
            y_a = sb.tile([P, 2, N], F32)
            y_b = sb.tile([P, 2, N], F32)
            b_pad = sb.tile([P, 2, 676], F32)
            q_pad = sb.tile([P, 4, N], F32)
            k_pad = sb.tile([P, 4, N], F32)
            v_aug = sb.tile([P, 5, 512], F32)
            attn_pad = sb.tile([P, 4, N], F32)
            a_psa = sb.tile([P, 2, N], F32)
            h_ffn = sb.tile([P, 4, N], F32)
            f_tmp = sb.tile([P, 2, N], F32)
            p_sb = sb.tile([P, 2, N], F32)
            e_sb = sb.tile([P, N], F32)
            e_pad = sb.tile([P, 676], F32)
            m_sb = sb.tile([P, 2, N], F32)
            diag_sb = sb.tile([P, 18, P], F32)

            # early prep that can hide under cv1/AR1: diag weights, v_aug init
            for mc in range(2):
                for t in range(9):
                    nc.vector.tensor_scalar(
                        diag_sb[:, mc * 9 + t, :], id128[:],
                        pew[:, mc, t:t + 1], None, op0=OP.mult)
            nc.vector.memset(v_aug[:], 0.0)
            nc.vector.memset(
                v_aug[:].rearrange("p a (h c) -> p a h c", c=32)[:, :, :, 0], 1.0)

            # =========== BN stat sync helper ===========
            def bn_sync(tag, nchunk, psums, raws, gate2=None):
                """psums: list of nchunk 2-bank psum tiles holding conv out.
                Computes local (mean, var) per chunk, builds [mu, var+mu^2]
                payload, AllGathers, reduces, returns (scale r, bias -mu*r)
                as [P, nchunk] tiles."""
                st = small.tile([P, nchunk, 2, 6], F32, tag=f"st{tag}")
                for mc in range(nchunk):
                    for nh in range(2):
                        nc.vector.bn_stats(
                            st[:, mc, nh, :],
                            psums[mc][:, nh * 512: nh * 512 + NH2])
                    nc.vector.tensor_copy(raws[mc], h3(psums[mc]))
                agg = small.tile([P, nchunk, 2], F32, tag=f"agg{tag}")
                for mc in range(nchunk):
                    nc.vector.bn_aggr(agg[:, mc, :],
                                      st[:, mc, :, :].rearrange("p a b -> p (a b)"))
                pay = small.tile([P, nchunk, 2], F32, tag=f"pay{tag}")
                # pay0 = mu, pay1 = var + mu^2   (optionally gated)
                nc.vector.tensor_tensor(pay[:, :, 1], agg[:, :, 0], agg[:, :, 0],
                                        op=OP.mult)
                nc.vector.tensor_tensor(pay[:, :, 1], pay[:, :, 1], agg[:, :, 1],
                                        op=OP.add)
                if gate2 is None:
                    nc.vector.tensor_copy(pay[:, :, 0], agg[:, :, 0])
                else:
                    gb, gb2 = gate2
                    nc.vector.tensor_scalar(pay[:, :, 0], agg[:, :, 0], gb[:],
                                            None, op0=OP.mult)
                    nc.vector.tensor_scalar(pay[:, :, 1], pay[:, :, 1], gb2[:],
                                            None, op0=OP.mult)
                bin_ = dram.tile([P, nchunk, 2], F32, tag=f"bin{tag}")
                bout = dram.tile([NCORES, P, nchunk, 2], F32, tag=f"bout{tag}")
                nc.sync.dma_start(bin_[:], pay[:])
                nc.gpsimd.collective_compute(
                    "AllGather", OP.bypass,
                    replica_groups=[list(range(NCORES))],
                    ins=[bin_[:].opt()], outs=[bout[:].opt()])
                gat = small.tile([P, nchunk, 2, NCORES], F32, tag=f"gat{tag}")
                nc.sync.dma_start(gat[:], bout[:].transpose([1, 2, 3, 0]))
                sums = small.tile([P, nchunk, 2, 1], F32, tag=f"sums{tag}")
                nc.vector.reduce_sum(sums[:], gat[:], axis=mybir.AxisListType.X)
                negmu = small.tile([P, nchunk], F32, tag=f"negmu{tag}")
                ex2e = small.tile([P, nchunk], F32, tag=f"ex2e{tag}")
                nc.vector.tensor_scalar(negmu[:], sums[:, :, 0, 0], -1.0 / NCORES,
                                        None, op0=OP.mult)
                nc.vector.tensor_scalar(ex2e[:], sums[:, :, 1, 0], 1.0 / NCORES,
                                        EPS, op0=OP.mult, op1=OP.add)
                var = small.tile([P, nchunk], F32, tag=f"var{tag}")
                nc.vector.tensor_tensor(var[:], negmu[:], negmu[:], op=OP.mult)
                nc.vector.tensor_tensor(var[:], ex2e[:], var[:], op=OP.subtract)
                lnv = small.tile([P, nchunk], F32, tag=f"lnv{tag}")
                nc.scalar.activation(lnv[:], var[:], AF.Ln)
                r = small.tile([P, nchunk], F32, tag=f"r{tag}")
                nc.scalar.activation(r[:], lnv[:], AF.Exp, scale=-0.5)
                nb = small.tile([P, nchunk], F32, tag=f"nb{tag}")
                nc.vector.tensor_tensor(nb[:], negmu[:], r[:], op=OP.mult)
                return r, nb

            def h3(t):
                """psum tile 3D view [p, 2, 288]."""
                return t[:].rearrange("p (a f) -> p a f", f=512)[:, :, 0:NH2]

            # =========== Phase 1: cv1 + BN1 + relu ===========
            cv1_ps = []
            for mc in range(4):
                pt = ps_conv.tile([P, 1024], F32, tag="conv")
                for nh in range(2):
                    for kc in range(4):
                        nc.tensor.matmul(
                            pt[:, nh * 512: nh * 512 + NH2],
                            w1t[:, kc, mc * P:(mc + 1) * P],
                            x_sb[:, kc, nh * NH2:(nh + 1) * NH2],
                            start=(kc == 0), stop=(kc == 3))
                cv1_ps.append(pt)
            cv1_raws = [y_a[:, 0, :], y_a[:, 1, :], y_b[:, 0, :], y_b[:, 1, :]]
            r1, nb1 = bn_sync(1, 4, cv1_ps,
                              [r.rearrange("p (a f) -> p a f", f=NH2)
                               for r in cv1_raws])
            # apply b-half first (chunks 2,3) so attention starts sooner
            for mc in (2, 3, 0, 1):
                buf = cv1_raws[mc]
                nc.scalar.activation(buf, buf, AF.Relu,
                                     bias=nb1[:, mc:mc + 1],
                                     scale=r1[:, mc:mc + 1])

            # b_pad for the depthwise positional conv
            nc.vector.memset(b_pad[:], 0.0)
            for mc in range(2):
                nc.vector.tensor_copy(
                    b_pad[:, mc, :].rearrange("p (h w) -> p h w", w=26)[:, 1:25, 1:25],
                    y_b[:, mc, :].rearrange("p (h w) -> p h w", w=24))

            # =========== Phase 2: attention ===========
            # qkv: q_pad / k_pad [P, 4, 576] (head-padded), v^T into v_aug
            for mc in range(4):
                ptq = ps_conv.tile([P, 1024], F32, tag="conv")
                for nh in range(2):
                    for kc in range(2):
                        nc.tensor.matmul(
                            ptq[:, nh * 512: nh * 512 + NH2],
                            wqt[:, kc, mc * P:(mc + 1) * P],
                            y_b[:, kc, nh * NH2:(nh + 1) * NH2],
                            start=(kc == 0), stop=(kc == 1))
                nc.vector.tensor_copy(
                    q_pad[:, mc, :].rearrange("p (a f) -> p a f", f=NH2), h3(ptq))
                ptk = ps_conv.tile([P, 1024], F32, tag="conv")
                for nh in range(2):
                    for kc in range(2):
                        nc.tensor.matmul(
                            ptk[:, nh * 512: nh * 512 + NH2],
                            wkt[:, kc, mc * P:(mc + 1) * P],
                            y_b[:, kc, nh * NH2:(nh + 1) * NH2],
                            start=(kc == 0), stop=(kc == 1))
                nc.scalar.activation(
                    k_pad[:, mc, :].rearrange("p (a f) -> p a f", f=NH2),
                    h3(ptk), AF.Copy)
            for mcv in range(5):
                cnt = MCNTS[mcv]
                ptv = ps_conv.tile([P, 256], F32, tag="conv")
                for kc in range(2):
                    nc.tensor.matmul(
                        ptv[0:cnt, :],
                        y_b[:, kc, mcv * P: mcv * P + cnt],
                        wvt[:, kc, :], start=(kc == 0), stop=(kc == 1))
                nc.vector.tensor_copy(
                    v_aug[0:cnt, mcv, :].rearrange("p (h c) -> p h c", c=32)[:, :, 1:17],
                    ptv[0:cnt, :].rearrange("p (h d) -> p h d", d=16))

            # per-group attention (4 heads per group, col-tiled AV)
            for g in range(4):
                av = ps_av.tile([P, 1024], F32, tag="av")
                for nh in range(2):
                    nc.tensor.matmul(av[:, nh * 512: nh * 512 + NH2],
                                     zc[:], zr[:], start=True, stop=False,
                                     skip_group_check=True)
                for j in range(4):
                    h = 4 * g + j
                    ch, off = h // 4, 32 * (h % 4)
                    for mcv in range(5):
                        cnt = MCNTS[mcv]
                        sp = ps_conv.tile([P, 1024], F32, tag="conv")
                        for nh in range(2):
                            nc.tensor.matmul(
                                sp[0:cnt, nh * 512: nh * 512 + NH2],
                                k_pad[off:off + 32, ch, mcv * P: mcv * P + cnt],
                                q_pad[off:off + 32, ch, nh * NH2:(nh + 1) * NH2],
                                start=True, stop=True, tile_position=(off, 0))
                        est = est_pool.tile([P, 2, NH2], F32, tag="est")
                        nc.scalar.activation(
                            est[0:cnt, :, :],
                            sp[0:cnt, :].rearrange("p (a f) -> p a f", f=512)[:, :, 0:NH2],
                            AF.Exp, scale=0.25)
                        for nh in range(2):
                            nc.tensor.matmul(
                                av[off:off + 32, nh * 512: nh * 512 + NH2],
                                v_aug[0:cnt, mcv, 32 * h:32 * h + 32],
                                est[0:cnt, nh, :],
                                start=False, stop=(mcv == 4),
                                tile_position=(0, off), skip_group_check=True)
                # normalize group: denom rows at 32j (ones-first layout)
                av_sb = avsb_pool.tile([P, 2, NH2], F32, tag="avsb")
                nc.vector.tensor_copy(av_sb[:], h3(av))
                den4 = small.tile([4, 2, NH2], F32, tag="den4")
                nc.sync.dma_start(den4[:], av_sb[0:P:32, :, :])
                rec4 = small.tile([4, 2, NH2], F32, tag="rec4")
                nc.vector.reciprocal(rec4[:], den4[:])
                for nh in range(2):
                    rb = ps_rb.tile([P, NH2], F32, tag="rb")
                    nc.tensor.matmul(rb[:], e4t[:], rec4[:, nh, :],
                                     start=True, stop=True)
                    nc.vector.tensor_tensor(
                        attn_pad[:, g, nh * NH2:(nh + 1) * NH2],
                        av_sb[:, nh, :], rb[:], op=OP.mult)

            # aproj + pe(depthwise) + shortcut -> a_psa
            for mc in range(2):
                pt = ps_conv.tile([P, 1024], F32, tag="conv")
                for nh in range(2):
                    for kc in range(4):
                        nc.tensor.matmul(
                            pt[:, nh * 512: nh * 512 + NH2],
                            wat[:, kc, mc * P:(mc + 1) * P],
                            attn_pad[:, kc, nh * NH2:(nh + 1) * NH2],
                            start=(kc == 0), stop=False)
                    for t in range(9):
                        u, v = t // 3, t % 3
                        win = b_pad[:, mc, :].rearrange("p (h w) -> p h w", w=26)[
                            :, u + nh * 12: u + nh * 12 + 12, v: v + 24]
                        nc.tensor.matmul(
                            pt[:, nh * 512: nh * 512 + NH2].rearrange(
                                "p (h w) -> p h w", w=24),
                            diag_sb[:, mc * 9 + t, :], win,
                            start=False, stop=(t == 8))
                nc.vector.tensor_tensor(
                    a_psa[:, mc, :].rearrange("p (a f) -> p a f", f=NH2),
                    h3(pt),
                    y_b[:, mc, :].rearrange("p (a f) -> p a f", f=NH2),
                    op=OP.add)

            # =========== Phase 3: ffn ===========
            ffn1_ps = []
            for mc in range(4):
                pt = ps_conv.tile([P, 1024], F32, tag="conv")
                for nh in range(2):
                    for kc in range(2):
                        nc.tensor.matmul(
                            pt[:, nh * 512: nh * 512 + NH2],
                            wf1t[:, kc, mc * P:(mc + 1) * P],
                            a_psa[:, kc, nh * NH2:(nh + 1) * NH2],
                            start=(kc == 0), stop=(kc == 1))
                ffn1_ps.append(pt)
            r2, nb2 = bn_sync(2, 4, ffn1_ps,
                              [h_ffn[:, mc, :].rearrange("p (a f) -> p a f",
                                                         f=NH2)
                               for mc in range(4)])
            for mc in range(4):
                buf = h_ffn[:, mc, :]
                nc.scalar.activation(buf, buf, AF.Relu,
                                     bias=nb2[:, mc:mc + 1],
                                     scale=r2[:, mc:mc + 1])

            ffn2_ps = []
            for mc in range(2):
                pt = ps_conv.tile([P, 1024], F32, tag="conv")
                for nh in range(2):
                    for kc in range(4):
                        nc.tensor.matmul(
                            pt[:, nh * 512: nh * 512 + NH2],
                            wf2t[:, kc, mc * P:(mc + 1) * P],
                            h_ffn[:, kc, nh * NH2:(nh + 1) * NH2],
                            start=(kc == 0), stop=(kc == 3))
                ffn2_ps.append(pt)
            # ec partial on a_psa overlaps AR3
            ec_pt = ps_conv.tile([P, 1024], F32, tag="conv")
            for nh in range(2):
                for kc in range(2):
                    nc.tensor.matmul(
                        ec_pt[:, nh * 512: nh * 512 + NH2],
                        wect[:, kc, :],
                        a_psa[:, kc, nh * NH2:(nh + 1) * NH2],
                        start=(kc == 0), stop=False)
            r3, nb3 = bn_sync(3, 2, ffn2_ps,
                              [f_tmp[:, mc, :].rearrange("p (a f) -> p a f",
                                                         f=NH2)
                               for mc in range(2)])
            for mc in range(2):
                buf = f_tmp[:, mc, :]
                nc.scalar.activation(buf, buf, AF.Identity,
                                     bias=nb3[:, mc:mc + 1],
                                     scale=r3[:, mc:mc + 1])
                nc.vector.tensor_tensor(p_sb[:, mc, :], f_tmp[:, mc, :],
                                        a_psa[:, mc, :], op=OP.add)

            # =========== Phase 4: msab ===========
            # finish ec conv with the f part, then BN4 + relu -> e
            for nh in range(2):
                for kc in range(2):
                    nc.tensor.matmul(
                        ec_pt[:, nh * 512: nh * 512 + NH2],
                        wect[:, kc, :],
                        f_tmp[:, kc, nh * NH2:(nh + 1) * NH2],
                        start=False, stop=(kc == 1))
            r4, nb4 = bn_sync(4, 1, [ec_pt],
                              [e_sb[:].rearrange("p (a f) -> p a f", f=NH2)])
            nc.scalar.activation(e_sb[:], e_sb[:], AF.Relu,
                                 bias=nb4[:, 0:1], scale=r4[:, 0:1])

            # gate = sigmoid(sum(e * g) / sqrt(128*576)); b_pad is dead, reuse
            acc_e = small.tile([P, 1], F32, tag="acc_e")
            nc.scalar.activation(b_pad[:, 0, 0:N], e_sb[:], AF.Copy,
                                 scale=gvec[:], accum_out=acc_e[:])
            gd_ps = ps_rb.tile([1, 1], F32, tag="rb")
            nc.tensor.matmul(gd_ps[:], ones_col[:], acc_e[:],
                             start=True, stop=True)
            sg = small.tile([1, 1], F32, tag="sg")
            nc.scalar.activation(sg[:], gd_ps[:], AF.Exp,
                                 scale=-1.0 / float(np.sqrt(128.0 * N)))
            sg1 = small.tile([1, 1], F32, tag="sg1")
            nc.vector.tensor_scalar(sg1[:], sg[:], 1.0, None, op0=OP.add)
            grec = small.tile([1, 1], F32, tag="grec")
            nc.vector.reciprocal(grec[:], sg1[:])
            gb_ps = ps_rb.tile([P, 1], F32, tag="rb")
            nc.tensor.matmul(gb_ps[:], ones_row[:], grec[:],
                             start=True, stop=True)
            gb = small.tile([P, 1], F32, tag="gb")
            nc.vector.tensor_copy(gb[:], gb_ps[:])
            gb2 = small.tile([P, 1], F32, tag="gb2")
            nc.vector.tensor_tensor(gb2[:], gb[:], gb[:], op=OP.mult)

            # e_pad + mproj (gate folded into BN via gated stats)
            nc.vector.memset(e_pad[:], 0.0)
            nc.vector.tensor_copy(
                e_pad[:].rearrange("p (h w) -> p h w", w=26)[:, 1:25, 1:25],
                e_sb[:].rearrange("p (h w) -> p h w", w=24))
            mp_ps = []
            for mc in range(2):
                pt = ps_conv.tile([P, 1024], F32, tag="conv")
                for nh in range(2):
                    for t in range(9):
                        u, v = t // 3, t % 3
                        win = e_pad[:].rearrange("p (h w) -> p h w", w=26)[
                            :, u + nh * 12: u + nh * 12 + 12, v: v + 24]
                        nc.tensor.matmul(
                            pt[:, nh * 512: nh * 512 + NH2].rearrange(
                                "p (h w) -> p h w", w=24),
                            wmpt[:, t, mc * P:(mc + 1) * P], win,
                            start=(t == 0), stop=(t == 8))
                mp_ps.append(pt)
            r5, nb5 = bn_sync(5, 2, mp_ps,
                              [m_sb[:, mc, :].rearrange("p (a f) -> p a f",
                                                        f=NH2)
                               for mc in range(2)], gate2=(gb, gb2))
            r5g = small.tile([P, 2], F32, tag="r5g")
            nc.vector.tensor_scalar(r5g[:], r5[:], gb[:], None, op0=OP.mult)
            for mc in range(2):
                buf = m_sb[:, mc, :]
                nc.scalar.activation(buf, buf, AF.Relu,
                                     bias=nb5[:, mc:mc + 1],
                                     scale=r5g[:, mc:mc + 1])

            # =========== Phase 5: cv2 over [a, b, p, m] ===========
            cat = [y_a[:, 0, :], y_a[:, 1, :], y_b[:, 0, :], y_b[:, 1, :],
                   p_sb[:, 0, :], p_sb[:, 1, :], m_sb[:, 0, :], m_sb[:, 1, :]]
            cv2_ps = []
            for mc in range(4):
                pt = ps_conv.tile([P, 1024], F32, tag="conv")
                for nh in range(2):
                    for kc in range(8):
                        nc.tensor.matmul(
                            pt[:, nh * 512: nh * 512 + NH2],
                            w2t[:, kc, mc * P:(mc + 1) * P],
                            cat[kc][:, nh * NH2:(nh + 1) * NH2],
                            start=(kc == 0), stop=(kc == 7))
                cv2_ps.append(pt)
            out_sb = sb.tile([P, 4, N], F32)
            r6, nb6 = bn_sync(6, 4, cv2_ps,
                              [out_sb[:, mc, :].rearrange("p (a f) -> p a f",
                                                          f=NH2)
                               for mc in range(4)])
            for mc in range(4):
                buf = out_sb[:, mc, :]
                nc.scalar.activation(buf, buf, AF.Relu,
                                     bias=nb6[:, mc:mc + 1],
                                     scale=r6[:, mc:mc + 1])
                nc.sync.dma_start(
                    out_d[:].rearrange("(a p) f -> p a f", p=P)[:, mc, :], buf)

    nc.compile()
    return nc


def get_nc():
    if "nc" not in _CACHE:
        _CACHE["nc"] = _build()
    return _CACHE["nc"]


def host_prep(inputs):
    """Fold modulus*cos(phase) weights, build padded/transposed layouts and
    the 8 per-core input maps."""
    def w(m, p):
        return (np.asarray(inputs[m], np.float32)
                * np.cos(np.asarray(inputs[p], np.float32)))

    x = np.asarray(inputs["x"], np.float32)          # (2, 512, 4, 24, 24)
    guide = np.asarray(inputs["guide"], np.float32)  # (2, 512)
    B, C1, Qd = x.shape[0], x.shape[1], x.shape[2]

    w1 = w("cv1_m", "cv1_p")[:, :, 0, 0]             # (512, 512) [co, ci]
    wqkv = w("qkv_m", "qkv_p")[:, :, 0, 0]           # (768, 256)
    wq, wk, wv = wqkv[0:256], wqkv[256:512], wqkv[512:768]
    wa = w("aproj_m", "aproj_p")[:, :, 0, 0]         # (256, 256)
    pe = w("pe_m", "pe_p")[:, 0, :, :].reshape(256, 9)
    wf1 = w("ffn1_m", "ffn1_p")[:, :, 0, 0]          # (512, 256)
    wf2 = w("ffn2_m", "ffn2_p")[:, :, 0, 0]          # (256, 512)
    wec = w("ec_m", "ec_p")[:, :, 0, 0]              # (128, 256)
    wmp = w("mproj_m", "mproj_p")                    # (256, 128, 3, 3)
    w2 = w("cv2_m", "cv2_p")[:, :, 0, 0]             # (512, 1024)
    gl_w = np.asarray(inputs["gl_w"], np.float32)
    gl_b = np.asarray(inputs["gl_b"], np.float32)
    gfull = guide @ gl_w.T + gl_b                    # (2, 128)

    # q/k head-padded [ci, co'=512]: col 32h+d = W[16h+d, ci], d<16
    def pad_qk(wm):
        out = np.zeros((256, 512), np.float32)
        for h in range(16):
            out[:, 32 * h:32 * h + 16] = wm[16 * h:16 * h + 16, :].T
        return out

    wqt = pad_qk(wq)
    wkt = pad_qk(wk)
    # aproj with padded ci' (ones-first attn layout: channel' 32h+1+d)
    wat_pad = np.zeros((512, 256), np.float32)
    for h in range(16):
        wat_pad[32 * h + 1:32 * h + 17, :] = wa[:, 16 * h:16 * h + 16].T
    e4 = np.zeros((4, 128), np.float32)
    for j in range(4):
        e4[j, 32 * j + 1:32 * j + 17] = 1.0

    shared = {
        "w1t": np.ascontiguousarray(w1.T),
        "wqt": wqt, "wkt": wkt,
        "wvt": np.ascontiguousarray(wv.T),
        "wat_pad": wat_pad,
        "pe_w": pe,
        "wf1t": np.ascontiguousarray(wf1.T),
        "wf2t": np.ascontiguousarray(wf2.T),
        "wect": np.ascontiguousarray(wec.T),
        "wmpt": np.ascontiguousarray(
            wmp.transpose(2, 3, 1, 0).reshape(9, 128, 256)),
        "w2t": np.ascontiguousarray(w2.T),
        "id128": np.eye(128, dtype=np.float32),
        "e4": e4,
    }
    in_maps = []
    for core in range(NCORES):
        b, q = core // Qd, core % Qd
        m = dict(shared)
        m["x_s"] = np.ascontiguousarray(x[b, :, q].reshape(C1, N))
        m["gvec"] = np.ascontiguousarray(gfull[b].reshape(P, 1))
        in_maps.append(m)
    return in_maps, (B, Qd)


def get_runner():
    """Cached sharded jitted executable over the 8 axon cores, mirroring
    bass2jax.run_bass_via_pjrt (which re-traces on every call)."""
    if "runner" in _CACHE:
        return _CACHE["runner"]
    import jax
    import numpy as _np
    from jax.sharding import Mesh, PartitionSpec
    from jax.experimental.shard_map import shard_map
    import concourse.mybir as mybir
    from concourse.bass2jax import (_bass_exec_p, partition_id_tensor,
                                    install_neuronx_cc_hook)

    nc = get_nc()
    install_neuronx_cc_hook()
    partition_name = (nc.partition_id_tensor.name
                      if nc.partition_id_tensor else None)
    in_names, out_names, out_avals, zero_outs = [], [], [], []
    for alloc in nc.m.functions[0].allocations:
        if not isinstance(alloc, mybir.MemoryLocationSet):
            continue
        name = alloc.memorylocations[0].name
        if alloc.kind == "ExternalInput":
            if name != partition_name:
                in_names.append(name)
        elif alloc.kind == "ExternalOutput":
            shape = tuple(alloc.tensor_shape)
            dtype = mybir.dt.np(alloc.dtype)
            out_names.append(name)
            out_avals.append(jax.core.ShapedArray(shape, dtype))
            zero_outs.append(_np.zeros(shape, dtype))
    n_params, n_outs = len(in_names), len(out_avals)
    all_in_names = list(in_names) + list(out_names)
    if partition_name is not None:
        all_in_names.append(partition_name)
    donate = tuple(range(n_params, n_params + n_outs))

    def _body(*args):
        operands = list(args)
        if partition_name is not None:
            operands.append(partition_id_tensor())
        outs = _bass_exec_p.bind(
            *operands,
            out_avals=tuple(out_avals),
            in_names=tuple(all_in_names),
            out_names=tuple(out_names),
            lowering_input_output_aliases=(),
            sim_require_finite=True,
            sim_require_nnan=True,
            nc=nc,
        )
        return tuple(outs)

    devices = jax.devices()[:NCORES]
    mesh = Mesh(_np.asarray(devices), ("core",))
    in_specs = (PartitionSpec("core"),) * (n_params + n_outs)
    out_specs = (PartitionSpec("core"),) * n_outs
    sharded = jax.jit(
        shard_map(_body, mesh=mesh, in_specs=in_specs, out_specs=out_specs,
                  check_rep=False),
        donate_argnums=donate, keep_unused=True)
    runner = {
        "fn": sharded, "mesh": mesh, "in_names": in_names,
        "out_names": out_names, "out_avals": out_avals,
        "zero_outs": zero_outs, "n_params": n_params,
    }
    _CACHE["runner"] = runner
    return runner


def run_cores(in_maps):
    import numpy as _np
    r = get_runner()
    concat_in = [
        _np.concatenate([_np.asarray(in_maps[c][name])[None]
                         for c in range(NCORES)], axis=0).reshape(
            NCORES * in_maps[0][name].shape[0], *in_maps[0][name].shape[1:])
        for name in r["in_names"]]
    concat_zeros = [
        _np.zeros((NCORES * z.shape[0], *z.shape[1:]), z.dtype)
        for z in r["zero_outs"]]
    out_arrs = r["fn"](*concat_in, *concat_zeros)
    outs = []
    for c in range(NCORES):
        outs.append({
            name: _np.asarray(out_arrs[i]).reshape(
                NCORES, *r["out_avals"][i].shape)[c]
            for i, name in enumerate(r["out_names"])})
    return outs


def kernel(**inputs):
    in_maps, (B, Qd) = host_prep(inputs)
    results = run_cores(in_maps)
    out = np.zeros((B, 512, Qd, 24, 24), np.float32)
    for core in range(NCORES):
        b, q = core // Qd, core % Qd
        out[b, :, q] = results[core]["out"].reshape(512, 24, 24)
    return out
